# revision 35
# baseline (speedup 1.0000x reference)
"""Trainium2 Bass kernel for ExpKernelModule (Hawkes positive-likelihood intensities).

out[b,i] = sum_{j<i} alpha[u,v]*beta[u,v]*exp(clip(-beta[u,v]*(t_i-t_j), -20, 0))
with u=ct[b,i], v=ct[b,j], alpha=softplus(log_alpha), beta=softplus(log_beta).

Device algorithm (one batch per core, data-parallel over B=8):
the exp argument  log(a*b) - beta*(t_i - t_j)  is a rank-64 bilinear form over
the (receiver, trigger) type one-hots:

  arg[i,j] = W1[v,i]*oh[v,j] + W2[v,i]*(t_j*oh[v,j])     (sum over v)
  W1[v,i] = C1[u_i,v] - B[u_i,v]*t_i,  W2[v,i] = B[u_i,v],  oh[v,j] = 1[ct_j==v]

Per 128-row tile, matmuls produce the full exp-argument block in PSUM; ScalarE
applies Exp with a fused accum_out row-sum. Row tile r only needs columns
[0, 128*(r+1)); the diagonal 128x128 block gets a -1e4 additive strict-lower
mask (VectorE) before Exp.

PE dtype: float16. Each fp32 operand is split into a hi/lo fp16 pair (22
effective mantissa bits); per-operand errors scale with term magnitude, and
large-magnitude args are exactly the dead ones (exp ~ 0). Two accumulating
matmuls per chunk cover all needed hi/lo cross products:
  mm1 K=128: [W1h, W1l, W2h, W2l] x [oh, oh, th*oh, th*oh]
  mm2 K=64:  [W2h, W2l]           x [tl*oh, tl*oh]
(Measured on HW: fp32-PSUM-accumulating matmuls retire at ~2 cyc/col for
bf16/fp16 alike — N-column stream time dominates and K is nearly free, so
fp16 costs the same as bf16 here and keeps fp32-level accuracy. Splitting
into more, narrower-K matmuls with row-group tile_position packing overlaps
streams but loses to the extra per-MM overhead. fp32 is 4 cyc/col; fp32r is
a 12-bit-mantissa mode.)
Measured end-to-end error vs the fp32 reference: ~7e-6 absmax-relative.
Host prep is O(L*D) index gathers only.
"""

import numpy as np

B_, L, D, P = 8, 2048, 32, 128
NT = L // P  # row tiles per batch
MASK_NEG = -1.0e4
MMW = 512  # moving-operand width per matmul (ISA limit for fp32 PSUM out)
MM_DTYPE = "float16"  # fp16 pairs: ~7e-6 err; "bfloat16" pairs: ~4e-4 err

_cached = {}


def _build_nc():
    import concourse.bass as bass  # noqa: F401
    import concourse.tile as tile
    from concourse import bacc, mybir

    f32 = mybir.dt.float32
    f16 = getattr(mybir.dt, MM_DTYPE)

    nc = bacc.Bacc("TRN2", target_bir_lowering=False, debug=False, enable_asserts=False, num_devices=8)
    wa_d = nc.dram_tensor("wa", (4 * D, L), f16, kind="ExternalInput").ap()
    ra_d = nc.dram_tensor("ra", (4 * D, L), f16, kind="ExternalInput").ap()
    wb_d = nc.dram_tensor("wb", (2 * D, L), f16, kind="ExternalInput").ap()
    rb_d = nc.dram_tensor("rb", (2 * D, L), f16, kind="ExternalInput").ap()
    m_d = nc.dram_tensor("m", (P, P), f32, kind="ExternalInput").ap()
    # out[p, r] = row-sum for global row i = 128*r + p; one contiguous DMA
    o_d = nc.dram_tensor("o", (P, NT), f32, kind="ExternalOutput").ap()

    with tile.TileContext(nc) as tc:
        with (
            tc.tile_pool(name="singles", bufs=1) as singles,
            tc.tile_pool(name="psum_v7", bufs=2, space="PSUM") as psum,
            tc.tile_pool(name="acc", bufs=4) as accp,
        ):
            # Interleave input DMAs in consumption order (512-col pieces),
            # spread across the two HWDGE queues (sync + scalar) for overlap.
            wa_sb = singles.tile([4 * D, L], f16)
            wb_sb = singles.tile([2 * D, L], f16)
            ra_sb = singles.tile([4 * D, L], f16)
            rb_sb = singles.tile([2 * D, L], f16)
            m_sb = singles.tile([P, P], f32)
            # First pieces go out on gpsimd (SWDGE): its queue is ready ~7us
            # before the HWDGE engines clear the framework preamble. These
            # transfers finish early, so the SWDGE drain at kernel end is free.
            sl0 = slice(0, 512)
            nc.gpsimd.dma_start(ra_sb[:, sl0], ra_d[:, sl0])
            nc.gpsimd.dma_start(wa_sb[:, sl0], wa_d[:, sl0])
            nc.gpsimd.dma_start(rb_sb[:, sl0], rb_d[:, sl0])
            nc.gpsimd.dma_start(wb_sb[:, sl0], wb_d[:, sl0])
            nc.gpsimd.dma_start(m_sb[:, :], m_d[:, :])
            for c0 in range(512, L, 512):
                sl = slice(c0, c0 + 512)
                nc.sync.dma_start(ra_sb[:, sl], ra_d[:, sl])
                nc.scalar.dma_start(wa_sb[:, sl], wa_d[:, sl])
                nc.sync.dma_start(rb_sb[:, sl], rb_d[:, sl])
                nc.scalar.dma_start(wb_sb[:, sl], wb_d[:, sl])

            acc = accp.tile([P, NT], f32)
            acc2 = accp.tile([P, 2], f32)
            # End on narrower tiles: the widest Exp calls run mid-stream where
            # ScalarE has slack instead of serializing after the last matmul.
            for rt in list(range(12)) + [15, 14, 13, 12]:
                ncols = P * (rt + 1)
                pt = psum.tile([P, L], f32)
                wsl = slice(rt * P, (rt + 1) * P)
                # all mm1 chunks first, then all mm2 chunks: consecutive PE
                # matmuls hit different PSUM banks, so fill overlaps drain
                # (same-bank accumulate pairs back-to-back serialize the PE).
                for c0 in range(0, ncols, MMW):
                    w_len = min(MMW, ncols - c0)
                    csl = slice(c0, c0 + w_len)
                    nc.tensor.matmul(
                        pt[:, csl], wa_sb[:, wsl], ra_sb[:, csl],
                        start=True, stop=False,
                    )
                for c0 in range(0, ncols, MMW):
                    w_len = min(MMW, ncols - c0)
                    csl = slice(c0, c0 + w_len)
                    nc.tensor.matmul(
                        pt[:, csl], wb_sb[:, wsl], rb_sb[:, csl],
                        start=False, stop=True,
                    )
                # strict-lower mask on the diagonal 128x128 block
                nc.vector.tensor_add(
                    pt[:, ncols - P : ncols], pt[:, ncols - P : ncols], m_sb[:, :]
                )
                if rt != 12:
                    nc.scalar.activation(
                        pt[:, :ncols],
                        pt[:, :ncols],
                        mybir.ActivationFunctionType.Exp,
                        accum_out=acc[:, rt : rt + 1],
                    )
                else:
                    # split the last-processed tile's Exp so its first half
                    # overlaps the final matmuls and the tail ACT is shorter
                    h = ncols // 2
                    nc.scalar.activation(
                        pt[:, :h], pt[:, :h],
                        mybir.ActivationFunctionType.Exp,
                        accum_out=acc2[:, 0:1],
                    )
                    nc.scalar.activation(
                        pt[:, h:ncols], pt[:, h:ncols],
                        mybir.ActivationFunctionType.Exp,
                        accum_out=acc2[:, 1:2],
                    )
                    nc.vector.tensor_add(
                        acc[:, rt : rt + 1], acc2[:, 0:1], acc2[:, 1:2]
                    )
            nc.sync.dma_start(o_d[:, :], acc[:, :])

    nc.compile()
    return nc


def _softplus(x):
    return np.log1p(np.exp(-np.abs(x))) + np.maximum(x, 0.0)


def _host_prep(time_points, event_types, log_alpha, log_beta):
    t = np.asarray(time_points).astype(np.float64)  # (B, L)
    u = np.asarray(event_types).astype(np.int64)  # (B, L)
    A = _softplus(np.asarray(log_alpha).astype(np.float64))
    Bt = _softplus(np.asarray(log_beta).astype(np.float64))
    C1 = np.log(A * Bt)  # (D, D)

    if MM_DTYPE == "float16":
        f16 = np.float16
    else:
        import ml_dtypes

        f16 = ml_dtypes.bfloat16
    W1 = np.transpose(C1[u], (0, 2, 1)) - np.transpose(Bt[u], (0, 2, 1)) * t[:, None, :]
    W2 = np.transpose(Bt[u], (0, 2, 1))  # (B, D, L)
    W1h = W1.astype(f16); W1l = (W1 - W1h.astype(np.float64)).astype(f16)
    W2h = W2.astype(f16); W2l = (W2 - W2h.astype(np.float64)).astype(f16)
    th = t.astype(f16); tl = (t - th.astype(np.float64)).astype(f16)
    oh = (u[:, None, :] == np.arange(D)[None, :, None])  # (B, D, L) bool

    WA = np.concatenate([W1h, W1l, W2h, W2l], axis=1)  # (B, 4D, L) f16
    RA = np.concatenate(
        [oh, oh,
         th.astype(np.float64)[:, None, :] * oh,
         th.astype(np.float64)[:, None, :] * oh], axis=1
    ).astype(f16)  # (B, 4D, L)
    WB = np.concatenate([W2h, W2l], axis=1)  # (B, 2D, L)
    tlo = tl.astype(np.float64)[:, None, :] * oh
    RB = np.concatenate([tlo, tlo], axis=1).astype(f16)  # (B, 2D, L)
    mask = np.triu(np.full((P, P), MASK_NEG, dtype=np.float32), k=0)
    return WA, RA, WB, RB, mask


def _run(inputs, trace=False):
    from concourse.bass_utils import run_bass_kernel_spmd

    WA, RA, WB, RB, mask = _host_prep(
        inputs["time_points"],
        inputs["event_types"],
        inputs["log_alpha"],
        inputs["log_beta"],
    )
    if "nc" not in _cached:
        _cached["nc"] = _build_nc()
    nc = _cached["nc"]

    in_maps = [
        {"wa": WA[b], "ra": RA[b], "wb": WB[b], "rb": RB[b], "m": mask}
        for b in range(B_)
    ]
    bres = run_bass_kernel_spmd(
        nc, in_maps, core_ids=list(range(B_)), trace=trace,
        trace_cores=[0] if trace else None,
    )
    # o is (P, NT) with out[i=128*r+p] = o[p, r]
    out = np.stack(
        [bres.results[b]["o"].reshape(P, NT).T.reshape(L) for b in range(B_)], axis=0
    )
    return out.astype(np.float32), bres


def kernel(**inputs) -> np.ndarray:
    out, _ = _run(inputs, trace=False)
    return out


# revision 38
# speedup vs baseline: 1.0291x; 1.0291x over previous
"""Trainium2 Bass kernel for ExpKernelModule (Hawkes positive-likelihood intensities).

out[b,i] = sum_{j<i} alpha[u,v]*beta[u,v]*exp(clip(-beta[u,v]*(t_i-t_j), -20, 0))
with u=ct[b,i], v=ct[b,j], alpha=softplus(log_alpha), beta=softplus(log_beta).

Device algorithm (one batch per core, data-parallel over B=8):
the exp argument  log(a*b) - beta*(t_i - t_j)  is a rank-64 bilinear form over
the (receiver, trigger) type one-hots:

  arg[i,j] = W1[v,i]*oh[v,j] + W2[v,i]*(t_j*oh[v,j])     (sum over v)
  W1[v,i] = C1[u_i,v] - B[u_i,v]*t_i,  W2[v,i] = B[u_i,v],  oh[v,j] = 1[ct_j==v]

Per 128-row tile, matmuls produce the full exp-argument block in PSUM; ScalarE
applies Exp with a fused accum_out row-sum. Row tile r only needs columns
[0, 128*(r+1)); the diagonal 128x128 block gets a -1e4 additive strict-lower
mask (VectorE) before Exp.

PE dtype: float16. Each fp32 operand is split into a hi/lo fp16 pair (22
effective mantissa bits); per-operand errors scale with term magnitude, and
large-magnitude args are exactly the dead ones (exp ~ 0). Two accumulating
matmuls per chunk cover all needed hi/lo cross products:
  mm1 K=128: [W1h, W1l, W2h, W2l] x [oh, oh, th*oh, th*oh]
  mm2 K=64:  [W2h, W2l]           x [tl*oh, tl*oh]
(Measured on HW: fp32-PSUM-accumulating matmuls retire at ~2 cyc/col for
bf16/fp16 alike — N-column stream time dominates and K is nearly free, so
fp16 costs the same as bf16 here and keeps fp32-level accuracy. Splitting
into more, narrower-K matmuls with row-group tile_position packing overlaps
streams but loses to the extra per-MM overhead. fp32 is 4 cyc/col; fp32r is
a 12-bit-mantissa mode.)
Measured end-to-end error vs the fp32 reference: ~7e-6 absmax-relative.
Host prep is O(L*D) index gathers only.
"""

import numpy as np

B_, L, D, P = 8, 2048, 32, 128
NT = L // P  # row tiles per batch
MASK_NEG = -1.0e4
MMW = 512  # moving-operand width per matmul (ISA limit for fp32 PSUM out)
MM_DTYPE = "float16"  # fp16 pairs: ~7e-6 err; "bfloat16" pairs: ~4e-4 err

_cached = {}


def _build_nc():
    import concourse.bass as bass  # noqa: F401
    import concourse.tile as tile
    from concourse import bacc, mybir

    f32 = mybir.dt.float32
    f16 = getattr(mybir.dt, MM_DTYPE)

    nc = bacc.Bacc("TRN2", target_bir_lowering=False, debug=False, enable_asserts=False, num_devices=8)
    wa_d = nc.dram_tensor("wa", (4 * D, L), f16, kind="ExternalInput").ap()
    ra_d = nc.dram_tensor("ra", (4 * D, L), f16, kind="ExternalInput").ap()
    wb_d = nc.dram_tensor("wb", (2 * D, L), f16, kind="ExternalInput").ap()
    rb_d = nc.dram_tensor("rb", (2 * D, L), f16, kind="ExternalInput").ap()
    m_d = nc.dram_tensor("m", (P, P), f32, kind="ExternalInput").ap()
    # out[p, r] = row-sum for global row i = 128*r + p; one contiguous DMA
    o_d = nc.dram_tensor("o", (P, NT), f32, kind="ExternalOutput").ap()

    with tile.TileContext(nc) as tc:
        with (
            tc.tile_pool(name="singles", bufs=1) as singles,
            tc.tile_pool(name="psum_v7", bufs=2, space="PSUM") as psum,
            tc.tile_pool(name="acc", bufs=4) as accp,
        ):
            # Interleave input DMAs in consumption order (512-col pieces),
            # spread across the two HWDGE queues (sync + scalar) for overlap.
            wa_sb = singles.tile([4 * D, L], f16)
            wb_sb = singles.tile([2 * D, L], f16)
            ra_sb = singles.tile([4 * D, L], f16)
            rb_sb = singles.tile([2 * D, L], f16)
            m_sb = singles.tile([P, P], f32)
            for c0 in range(0, L, 512):
                sl = slice(c0, c0 + 512)
                nc.sync.dma_start(ra_sb[:, sl], ra_d[:, sl])
                nc.scalar.dma_start(wa_sb[:, sl], wa_d[:, sl])
                nc.sync.dma_start(rb_sb[:, sl], rb_d[:, sl])
                nc.scalar.dma_start(wb_sb[:, sl], wb_d[:, sl])
                if c0 == 0:
                    nc.scalar.dma_start(m_sb[:, :], m_d[:, :])

            acc = accp.tile([P, NT], f32)
            acc2 = accp.tile([P, 2], f32)
            for rt in range(NT):
                ncols = P * (rt + 1)
                pt = psum.tile([P, L], f32)
                wsl = slice(rt * P, (rt + 1) * P)
                # all mm1 chunks first, then all mm2 chunks: consecutive PE
                # matmuls hit different PSUM banks, so fill overlaps drain
                # (same-bank accumulate pairs back-to-back serialize the PE).
                for c0 in range(0, ncols, MMW):
                    w_len = min(MMW, ncols - c0)
                    csl = slice(c0, c0 + w_len)
                    nc.tensor.matmul(
                        pt[:, csl], wa_sb[:, wsl], ra_sb[:, csl],
                        start=True, stop=False,
                    )
                for c0 in range(0, ncols, MMW):
                    w_len = min(MMW, ncols - c0)
                    csl = slice(c0, c0 + w_len)
                    nc.tensor.matmul(
                        pt[:, csl], wb_sb[:, wsl], rb_sb[:, csl],
                        start=False, stop=True,
                    )
                # strict-lower mask on the diagonal 128x128 block
                nc.vector.tensor_add(
                    pt[:, ncols - P : ncols], pt[:, ncols - P : ncols], m_sb[:, :]
                )
                if rt < NT - 1:
                    nc.scalar.activation(
                        pt[:, :ncols],
                        pt[:, :ncols],
                        mybir.ActivationFunctionType.Exp,
                        accum_out=acc[:, rt : rt + 1],
                    )
                else:
                    # split the last (widest) Exp in two so its first half
                    # overlaps the final matmuls and the tail ACT is shorter
                    h = ncols // 2
                    nc.scalar.activation(
                        pt[:, :h], pt[:, :h],
                        mybir.ActivationFunctionType.Exp,
                        accum_out=acc2[:, 0:1],
                    )
                    nc.scalar.activation(
                        pt[:, h:ncols], pt[:, h:ncols],
                        mybir.ActivationFunctionType.Exp,
                        accum_out=acc2[:, 1:2],
                    )
                    nc.vector.tensor_add(
                        acc[:, rt : rt + 1], acc2[:, 0:1], acc2[:, 1:2]
                    )
            nc.sync.dma_start(o_d[:, :], acc[:, :])

    nc.compile()
    return nc


def _softplus(x):
    return np.log1p(np.exp(-np.abs(x))) + np.maximum(x, 0.0)


def _host_prep(time_points, event_types, log_alpha, log_beta):
    t = np.asarray(time_points).astype(np.float64)  # (B, L)
    u = np.asarray(event_types).astype(np.int64)  # (B, L)
    A = _softplus(np.asarray(log_alpha).astype(np.float64))
    Bt = _softplus(np.asarray(log_beta).astype(np.float64))
    C1 = np.log(A * Bt)  # (D, D)

    if MM_DTYPE == "float16":
        f16 = np.float16
    else:
        import ml_dtypes

        f16 = ml_dtypes.bfloat16
    W1 = np.transpose(C1[u], (0, 2, 1)) - np.transpose(Bt[u], (0, 2, 1)) * t[:, None, :]
    W2 = np.transpose(Bt[u], (0, 2, 1))  # (B, D, L)
    W1h = W1.astype(f16); W1l = (W1 - W1h.astype(np.float64)).astype(f16)
    W2h = W2.astype(f16); W2l = (W2 - W2h.astype(np.float64)).astype(f16)
    th = t.astype(f16); tl = (t - th.astype(np.float64)).astype(f16)
    oh = (u[:, None, :] == np.arange(D)[None, :, None])  # (B, D, L) bool

    WA = np.concatenate([W1h, W1l, W2h, W2l], axis=1)  # (B, 4D, L) f16
    RA = np.concatenate(
        [oh, oh,
         th.astype(np.float64)[:, None, :] * oh,
         th.astype(np.float64)[:, None, :] * oh], axis=1
    ).astype(f16)  # (B, 4D, L)
    WB = np.concatenate([W2h, W2l], axis=1)  # (B, 2D, L)
    tlo = tl.astype(np.float64)[:, None, :] * oh
    RB = np.concatenate([tlo, tlo], axis=1).astype(f16)  # (B, 2D, L)
    mask = np.triu(np.full((P, P), MASK_NEG, dtype=np.float32), k=0)
    return WA, RA, WB, RB, mask


def _run(inputs, trace=False):
    from concourse.bass_utils import run_bass_kernel_spmd

    WA, RA, WB, RB, mask = _host_prep(
        inputs["time_points"],
        inputs["event_types"],
        inputs["log_alpha"],
        inputs["log_beta"],
    )
    if "nc" not in _cached:
        _cached["nc"] = _build_nc()
    nc = _cached["nc"]

    in_maps = [
        {"wa": WA[b], "ra": RA[b], "wb": WB[b], "rb": RB[b], "m": mask}
        for b in range(B_)
    ]
    bres = run_bass_kernel_spmd(
        nc, in_maps, core_ids=list(range(B_)), trace=trace,
        trace_cores=[0] if trace else None,
    )
    # o is (P, NT) with out[i=128*r+p] = o[p, r]
    out = np.stack(
        [bres.results[b]["o"].reshape(P, NT).T.reshape(L) for b in range(B_)], axis=0
    )
    return out.astype(np.float32), bres


def kernel(**inputs) -> np.ndarray:
    out, _ = _run(inputs, trace=False)
    return out


# revision 39
# speedup vs baseline: 1.0722x; 1.0419x over previous
"""Trainium2 Bass kernel for ExpKernelModule (Hawkes positive-likelihood intensities).

out[b,i] = sum_{j<i} alpha[u,v]*beta[u,v]*exp(clip(-beta[u,v]*(t_i-t_j), -20, 0))
with u=ct[b,i], v=ct[b,j], alpha=softplus(log_alpha), beta=softplus(log_beta).

Device algorithm (one batch per core, data-parallel over B=8):
the exp argument  log(a*b) - beta*(t_i - t_j)  is a rank-64 bilinear form over
the (receiver, trigger) type one-hots:

  arg[i,j] = W1[v,i]*oh[v,j] + W2[v,i]*(t_j*oh[v,j])     (sum over v)
  W1[v,i] = C1[u_i,v] - B[u_i,v]*t_i,  W2[v,i] = B[u_i,v],  oh[v,j] = 1[ct_j==v]

Per 128-row tile, matmuls produce the full exp-argument block in PSUM; ScalarE
applies Exp with a fused accum_out row-sum. Row tile r only needs columns
[0, 128*(r+1)); the diagonal 128x128 block gets a -1e4 additive strict-lower
mask (VectorE) before Exp.

PE dtype: float16. Each fp32 operand is split into a hi/lo fp16 pair (22
effective mantissa bits); per-operand errors scale with term magnitude, and
large-magnitude args are exactly the dead ones (exp ~ 0). Two accumulating
matmuls per chunk cover all needed hi/lo cross products:
  mm1 K=128: [W1h, W1l, W2h, W2l] x [oh, oh, th*oh, th*oh]
  mm2 K=64:  [W2h, W2l]           x [tl*oh, tl*oh]
(Measured on HW: fp32-PSUM-accumulating matmuls retire at ~2 cyc/col for
bf16/fp16 alike — N-column stream time dominates and K is nearly free, so
fp16 costs the same as bf16 here and keeps fp32-level accuracy. Splitting
into more, narrower-K matmuls with row-group tile_position packing overlaps
streams but loses to the extra per-MM overhead. fp32 is 4 cyc/col; fp32r is
a 12-bit-mantissa mode.)
Measured end-to-end error vs the fp32 reference: ~7e-6 absmax-relative.
Host prep is O(L*D) index gathers only.
"""

import numpy as np

B_, L, D, P = 8, 2048, 32, 128
NT = L // P  # row tiles per batch
MASK_NEG = -1.0e4
MMW = 512  # moving-operand width per matmul (ISA limit for fp32 PSUM out)
MM_DTYPE = "float16"  # fp16 pairs: ~7e-6 err; "bfloat16" pairs: ~4e-4 err

_cached = {}


def _build_nc():
    import concourse.bass as bass  # noqa: F401
    import concourse.tile as tile
    from concourse import bacc, mybir

    f32 = mybir.dt.float32
    f16 = getattr(mybir.dt, MM_DTYPE)

    nc = bacc.Bacc("TRN2", target_bir_lowering=False, debug=False, enable_asserts=False, num_devices=8)
    wa_d = nc.dram_tensor("wa", (4 * D, L), f16, kind="ExternalInput").ap()
    ra_d = nc.dram_tensor("ra", (4 * D, L), f16, kind="ExternalInput").ap()
    wb_d = nc.dram_tensor("wb", (2 * D, L), f16, kind="ExternalInput").ap()
    rb_d = nc.dram_tensor("rb", (2 * D, L), f16, kind="ExternalInput").ap()
    m_d = nc.dram_tensor("m", (P, P), f32, kind="ExternalInput").ap()
    # out[p, r] = row-sum for global row i = 128*r + p; one contiguous DMA
    o_d = nc.dram_tensor("o", (P, NT), f32, kind="ExternalOutput").ap()

    with tile.TileContext(nc) as tc:
        with (
            tc.tile_pool(name="singles", bufs=1) as singles,
            tc.tile_pool(name="psum_v7", bufs=2, space="PSUM") as psum,
            tc.tile_pool(name="acc", bufs=4) as accp,
        ):
            # Interleave input DMAs in consumption order (512-col pieces),
            # spread across the two HWDGE queues (sync + scalar) for overlap.
            wa_sb = singles.tile([4 * D, L], f16)
            wb_sb = singles.tile([2 * D, L], f16)
            ra_sb = singles.tile([4 * D, L], f16)
            rb_sb = singles.tile([2 * D, L], f16)
            m_sb = singles.tile([P, P], f32)
            for c0 in range(0, L, 512):
                sl = slice(c0, c0 + 512)
                nc.sync.dma_start(ra_sb[:, sl], ra_d[:, sl])
                nc.scalar.dma_start(wa_sb[:, sl], wa_d[:, sl])
                nc.sync.dma_start(rb_sb[:, sl], rb_d[:, sl])
                nc.scalar.dma_start(wb_sb[:, sl], wb_d[:, sl])
                if c0 == 0:
                    nc.scalar.dma_start(m_sb[:, :], m_d[:, :])

            bias0 = singles.tile([P, 1], f32)
            nc.vector.memset(bias0[:, :], 0.0)
            acc = accp.tile([P, NT], f32)
            for rt in range(NT):
                ncols = P * (rt + 1)
                pt = psum.tile([P, L], f32)
                wsl = slice(rt * P, (rt + 1) * P)
                # all mm1 chunks first, then all mm2 chunks: consecutive PE
                # matmuls hit different PSUM banks, so fill overlaps drain
                # (same-bank accumulate pairs back-to-back serialize the PE).
                for c0 in range(0, ncols, MMW):
                    w_len = min(MMW, ncols - c0)
                    csl = slice(c0, c0 + w_len)
                    nc.tensor.matmul(
                        pt[:, csl], wa_sb[:, wsl], ra_sb[:, csl],
                        start=True, stop=False,
                    )
                for c0 in range(0, ncols, MMW):
                    w_len = min(MMW, ncols - c0)
                    csl = slice(c0, c0 + w_len)
                    nc.tensor.matmul(
                        pt[:, csl], wb_sb[:, wsl], rb_sb[:, csl],
                        start=False, stop=True,
                    )
                # strict-lower mask on the diagonal 128x128 block
                nc.vector.tensor_add(
                    pt[:, ncols - P : ncols], pt[:, ncols - P : ncols], m_sb[:, :]
                )
                nc.scalar.activation(
                    pt[:, :ncols],
                    pt[:, :ncols],
                    mybir.ActivationFunctionType.Exp,
                    bias=bias0[:, :],
                    accum_out=acc[:, rt : rt + 1],
                )
            nc.sync.dma_start(o_d[:, :], acc[:, :])

    nc.compile()
    return nc


def _softplus(x):
    return np.log1p(np.exp(-np.abs(x))) + np.maximum(x, 0.0)


def _host_prep(time_points, event_types, log_alpha, log_beta):
    t = np.asarray(time_points).astype(np.float64)  # (B, L)
    u = np.asarray(event_types).astype(np.int64)  # (B, L)
    A = _softplus(np.asarray(log_alpha).astype(np.float64))
    Bt = _softplus(np.asarray(log_beta).astype(np.float64))
    C1 = np.log(A * Bt)  # (D, D)

    if MM_DTYPE == "float16":
        f16 = np.float16
    else:
        import ml_dtypes

        f16 = ml_dtypes.bfloat16
    W1 = np.transpose(C1[u], (0, 2, 1)) - np.transpose(Bt[u], (0, 2, 1)) * t[:, None, :]
    W2 = np.transpose(Bt[u], (0, 2, 1))  # (B, D, L)
    W1h = W1.astype(f16); W1l = (W1 - W1h.astype(np.float64)).astype(f16)
    W2h = W2.astype(f16); W2l = (W2 - W2h.astype(np.float64)).astype(f16)
    th = t.astype(f16); tl = (t - th.astype(np.float64)).astype(f16)
    oh = (u[:, None, :] == np.arange(D)[None, :, None])  # (B, D, L) bool

    WA = np.concatenate([W1h, W1l, W2h, W2l], axis=1)  # (B, 4D, L) f16
    RA = np.concatenate(
        [oh, oh,
         th.astype(np.float64)[:, None, :] * oh,
         th.astype(np.float64)[:, None, :] * oh], axis=1
    ).astype(f16)  # (B, 4D, L)
    WB = np.concatenate([W2h, W2l], axis=1)  # (B, 2D, L)
    tlo = tl.astype(np.float64)[:, None, :] * oh
    RB = np.concatenate([tlo, tlo], axis=1).astype(f16)  # (B, 2D, L)
    mask = np.triu(np.full((P, P), MASK_NEG, dtype=np.float32), k=0)
    return WA, RA, WB, RB, mask


def _run(inputs, trace=False):
    from concourse.bass_utils import run_bass_kernel_spmd

    WA, RA, WB, RB, mask = _host_prep(
        inputs["time_points"],
        inputs["event_types"],
        inputs["log_alpha"],
        inputs["log_beta"],
    )
    if "nc" not in _cached:
        _cached["nc"] = _build_nc()
    nc = _cached["nc"]

    in_maps = [
        {"wa": WA[b], "ra": RA[b], "wb": WB[b], "rb": RB[b], "m": mask}
        for b in range(B_)
    ]
    bres = run_bass_kernel_spmd(
        nc, in_maps, core_ids=list(range(B_)), trace=trace,
        trace_cores=[0] if trace else None,
    )
    # o is (P, NT) with out[i=128*r+p] = o[p, r]
    out = np.stack(
        [bres.results[b]["o"].reshape(P, NT).T.reshape(L) for b in range(B_)], axis=0
    )
    return out.astype(np.float32), bres


def kernel(**inputs) -> np.ndarray:
    out, _ = _run(inputs, trace=False)
    return out


# revision 41
# speedup vs baseline: 1.1579x; 1.0799x over previous
"""Trainium2 Bass kernel for ExpKernelModule (Hawkes positive-likelihood intensities).

out[b,i] = sum_{j<i} alpha[u,v]*beta[u,v]*exp(clip(-beta[u,v]*(t_i-t_j), -20, 0))
with u=ct[b,i], v=ct[b,j], alpha=softplus(log_alpha), beta=softplus(log_beta).

Device algorithm (one batch per core, data-parallel over B=8):
the exp argument  log(a*b) - beta*(t_i - t_j)  is a rank-64 bilinear form over
the (receiver, trigger) type one-hots:

  arg[i,j] = W1[v,i]*oh[v,j] + W2[v,i]*(t_j*oh[v,j])     (sum over v)
  W1[v,i] = C1[u_i,v] - B[u_i,v]*t_i,  W2[v,i] = B[u_i,v],  oh[v,j] = 1[ct_j==v]

Per 128-row tile, matmuls produce the full exp-argument block in PSUM; ScalarE
applies Exp with a fused accum_out row-sum. Row tile r only needs columns
[0, 128*(r+1)); the diagonal 128x128 block gets a -1e4 additive strict-lower
mask (VectorE) before Exp.

PE dtype: float16. Each fp32 operand is split into a hi/lo fp16 pair (22
effective mantissa bits); per-operand errors scale with term magnitude, and
large-magnitude args are exactly the dead ones (exp ~ 0). Two accumulating
matmuls per chunk cover all needed hi/lo cross products:
  mm1 K=128: [W1h, W1l, W2h, W2l] x [oh, oh, th*oh, th*oh]
  mm2 K=64:  [W2h, W2l]           x [tl*oh, tl*oh]
(Measured on HW: each matmul costs ~(398+N)/2.4GHz warm — 1 cyc/col stream
plus ~166ns of non-overlapped issue/drain (the per-MM LDWEIGHTS blocks
fill-after-fill pipelining and walrus's LDW dedup is unusable) — identically
for bf16/fp16, and K is nearly free, so fp16 costs the same as bf16 and keeps
fp32-level accuracy. Splitting into narrower-K matmuls with row-group
tile_position packing overlaps streams but loses to the extra per-MM
overhead. fp32 is 4 cyc/col; fp32r is a 12-bit-mantissa mode.)
Measured end-to-end error vs the fp32 reference: ~7e-6 absmax-relative.
Host prep is O(L*D) index gathers only.
"""

import numpy as np

B_, L, D, P = 8, 2048, 32, 128
NT = L // P  # row tiles per batch
MASK_NEG = -1.0e4
MMW = 512  # moving-operand width per matmul (ISA limit for fp32 PSUM out)
MM_DTYPE = "float16"  # fp16 pairs: ~7e-6 err; "bfloat16" pairs: ~4e-4 err

_cached = {}


def _build_nc():
    import concourse.bass as bass  # noqa: F401
    import concourse.tile as tile
    from concourse import bacc, mybir

    f32 = mybir.dt.float32
    f16 = getattr(mybir.dt, MM_DTYPE)

    nc = bacc.Bacc("TRN2", target_bir_lowering=False, debug=False, enable_asserts=False, num_devices=8)
    wa_d = nc.dram_tensor("wa", (4 * D, L), f16, kind="ExternalInput").ap()
    ra_d = nc.dram_tensor("ra", (4 * D, L), f16, kind="ExternalInput").ap()
    wb_d = nc.dram_tensor("wb", (2 * D, L), f16, kind="ExternalInput").ap()
    rb_d = nc.dram_tensor("rb", (2 * D, L), f16, kind="ExternalInput").ap()
    m_d = nc.dram_tensor("m", (P, P), f32, kind="ExternalInput").ap()
    # out[p, r] = row-sum for global row i = 128*r + p; one contiguous DMA
    o_d = nc.dram_tensor("o", (P, NT), f32, kind="ExternalOutput").ap()

    with tile.TileContext(nc) as tc:
        with (
            tc.tile_pool(name="singles", bufs=1) as singles,
            tc.tile_pool(name="psum_v7", bufs=2, space="PSUM") as psum,
            tc.tile_pool(name="acc", bufs=4) as accp,
        ):
            # Interleave input DMAs in consumption order (512-col pieces),
            # spread across the two HWDGE queues (sync + scalar) for overlap.
            wa_sb = singles.tile([4 * D, L], f16)
            wb_sb = singles.tile([2 * D, L], f16)
            ra_sb = singles.tile([4 * D, L], f16)
            rb_sb = singles.tile([2 * D, L], f16)
            m_sb = singles.tile([P, P], f32)
            # mm1 operands (ra/wa) ship one piece ahead of mm2's (rb/wb):
            # a tile's mm2 matmuls always trail its mm1s, so rb/wb can lag.
            def piece(eng, sb, dram, c0):
                sl = slice(c0, c0 + 512)
                eng.dma_start(sb[:, sl], dram[:, sl])

            piece(nc.sync, ra_sb, ra_d, 0)
            piece(nc.scalar, wa_sb, wa_d, 0)
            piece(nc.sync, ra_sb, ra_d, 512)
            piece(nc.scalar, wa_sb, wa_d, 512)
            piece(nc.sync, rb_sb, rb_d, 0)
            piece(nc.scalar, wb_sb, wb_d, 0)
            nc.scalar.dma_start(m_sb[:, :], m_d[:, :])
            for c0 in (1024, 1536):
                piece(nc.sync, ra_sb, ra_d, c0)
                piece(nc.scalar, wa_sb, wa_d, c0)
            for c0 in (512, 1024, 1536):
                piece(nc.sync, rb_sb, rb_d, c0)
                piece(nc.scalar, wb_sb, wb_d, c0)

            bias0 = singles.tile([P, 1], f32)
            nc.vector.memset(bias0[:, :], 0.0)
            acc = accp.tile([P, NT], f32)
            for rt in range(NT):
                ncols = P * (rt + 1)
                pt = psum.tile([P, L], f32)
                wsl = slice(rt * P, (rt + 1) * P)
                # all mm1 chunks first, then all mm2 chunks: consecutive PE
                # matmuls hit different PSUM banks, so fill overlaps drain
                # (same-bank accumulate pairs back-to-back serialize the PE).
                for c0 in range(0, ncols, MMW):
                    w_len = min(MMW, ncols - c0)
                    csl = slice(c0, c0 + w_len)
                    nc.tensor.matmul(
                        pt[:, csl], wa_sb[:, wsl], ra_sb[:, csl],
                        start=True, stop=False,
                    )
                for c0 in range(0, ncols, MMW):
                    w_len = min(MMW, ncols - c0)
                    csl = slice(c0, c0 + w_len)
                    nc.tensor.matmul(
                        pt[:, csl], wb_sb[:, wsl], rb_sb[:, csl],
                        start=False, stop=True,
                    )
                # strict-lower mask on the diagonal 128x128 block
                nc.vector.tensor_add(
                    pt[:, ncols - P : ncols], pt[:, ncols - P : ncols], m_sb[:, :]
                )
                nc.scalar.activation(
                    pt[:, :ncols],
                    pt[:, :ncols],
                    mybir.ActivationFunctionType.Exp,
                    bias=bias0[:, :],
                    accum_out=acc[:, rt : rt + 1],
                )
            nc.sync.dma_start(o_d[:, :], acc[:, :])

    nc.compile()
    return nc


def _softplus(x):
    return np.log1p(np.exp(-np.abs(x))) + np.maximum(x, 0.0)


def _host_prep(time_points, event_types, log_alpha, log_beta):
    t = np.asarray(time_points).astype(np.float64)  # (B, L)
    u = np.asarray(event_types).astype(np.int64)  # (B, L)
    A = _softplus(np.asarray(log_alpha).astype(np.float64))
    Bt = _softplus(np.asarray(log_beta).astype(np.float64))
    C1 = np.log(A * Bt)  # (D, D)

    if MM_DTYPE == "float16":
        f16 = np.float16
    else:
        import ml_dtypes

        f16 = ml_dtypes.bfloat16
    W1 = np.transpose(C1[u], (0, 2, 1)) - np.transpose(Bt[u], (0, 2, 1)) * t[:, None, :]
    W2 = np.transpose(Bt[u], (0, 2, 1))  # (B, D, L)
    W1h = W1.astype(f16); W1l = (W1 - W1h.astype(np.float64)).astype(f16)
    W2h = W2.astype(f16); W2l = (W2 - W2h.astype(np.float64)).astype(f16)
    th = t.astype(f16); tl = (t - th.astype(np.float64)).astype(f16)
    oh = (u[:, None, :] == np.arange(D)[None, :, None])  # (B, D, L) bool

    WA = np.concatenate([W1h, W1l, W2h, W2l], axis=1)  # (B, 4D, L) f16
    RA = np.concatenate(
        [oh, oh,
         th.astype(np.float64)[:, None, :] * oh,
         th.astype(np.float64)[:, None, :] * oh], axis=1
    ).astype(f16)  # (B, 4D, L)
    WB = np.concatenate([W2h, W2l], axis=1)  # (B, 2D, L)
    tlo = tl.astype(np.float64)[:, None, :] * oh
    RB = np.concatenate([tlo, tlo], axis=1).astype(f16)  # (B, 2D, L)
    mask = np.triu(np.full((P, P), MASK_NEG, dtype=np.float32), k=0)
    return WA, RA, WB, RB, mask


def _run(inputs, trace=False):
    from concourse.bass_utils import run_bass_kernel_spmd

    WA, RA, WB, RB, mask = _host_prep(
        inputs["time_points"],
        inputs["event_types"],
        inputs["log_alpha"],
        inputs["log_beta"],
    )
    if "nc" not in _cached:
        _cached["nc"] = _build_nc()
    nc = _cached["nc"]

    in_maps = [
        {"wa": WA[b], "ra": RA[b], "wb": WB[b], "rb": RB[b], "m": mask}
        for b in range(B_)
    ]
    bres = run_bass_kernel_spmd(
        nc, in_maps, core_ids=list(range(B_)), trace=trace,
        trace_cores=[0] if trace else None,
    )
    # o is (P, NT) with out[i=128*r+p] = o[p, r]
    out = np.stack(
        [bres.results[b]["o"].reshape(P, NT).T.reshape(L) for b in range(B_)], axis=0
    )
    return out.astype(np.float32), bres


def kernel(**inputs) -> np.ndarray:
    out, _ = _run(inputs, trace=False)
    return out
